# revision 20
# baseline (speedup 1.0000x reference)
"""Trainium2 Bass kernel for the DPL SAC-SMA hydrological model.

Strategy (per NeuronCore, 8 cores, units data-parallel, basins co-located):
  - LSTM parameter net: For_i over T, block-diagonal weights so both
    128-unit halves stay partition-aligned ([128,256] tiles throughout).
  - PET + SNOW17: reformulated as big elementwise ops + first-order
    recurrences evaluated with tensor_tensor_scan (ati/wi/wq scans).
  - SAC-SMA: For_i day loop, 4 unrolled sub-steps of fused DVE ops on
    [128,4] state tiles (units packed 128 partitions x 4 groups).
  - Routing: 15-lag causal conv via scalar_tensor_tensor accumulation.
  - Segment sums: one-hot matmuls (basins co-located per core).
"""
import sys

sys.path.insert(0, '/opt/trn_rl_repo')

import numpy as np

import bass_rust
import concourse.bass as bass
import concourse.mybir as mybir
from concourse import bass_utils
from concourse.tile import TileContext
from concourse.vector_clock import ScopedClock

f32 = np.float32
AOT = mybir.AluOpType
AF = mybir.ActivationFunctionType
DT = mybir.dt.float32

HID = 64
UH_LEN = 15
N_INC = 4
NP = 30
P = 128
G = 4
U = P * G          # 512 padded units per core
NCORES = 8
B_PAD = 64         # padded local basin count

PARAM_SPECS = [
    ("HAMON_COEF", 0.5, 2.0),
    ("SCF", 0.7, 1.4), ("PXTEMP", -2.0, 2.0), ("MFMAX", 0.5, 2.0), ("MFMIN", 0.05, 0.49),
    ("UADJ", 0.03, 0.19), ("MBASE", 0.0, 1.0), ("TIPM", 0.1, 1.0), ("PLWHC", 0.02, 0.3),
    ("NMF", 0.05, 0.5), ("DAYGM", 0.0, 0.3),
    ("UZTWM", 10.0, 300.0), ("UZFWM", 5.0, 150.0), ("LZTWM", 10.0, 500.0), ("LZFPM", 10.0, 1000.0),
    ("LZFSM", 5.0, 400.0), ("UZK", 0.1, 0.75), ("LZPK", 0.001, 0.05), ("LZSK", 0.01, 0.35),
    ("ZPERC", 5.0, 350.0), ("REXP", 1.0, 5.0), ("PFREE", 0.0, 0.8), ("PCTIM", 0.0, 0.1),
    ("ADIMP", 0.0, 0.4), ("RIVA", 0.0, 0.2), ("SIDE", 0.0, 0.5), ("RSERV", 0.0, 0.4),
    ("THETA_C", 0.0, 1.0), ("UH_N", 1.0, 6.0), ("UH_TAU", 0.5, 5.0),
]
PIDX = {name: k for k, (name, _, _) in enumerate(PARAM_SPECS)}


# --------------------------------------------------------------------------
# Workaround: this walrus build accepts only ONE sync-wait per instruction.
# --------------------------------------------------------------------------
def _split_multi_waits(nc):
    for fn in nc.m.functions:
        for bb in fn.blocks:
            insts = list(bb.instructions)
            out = []
            changed = False
            for inst in insts:
                si = inst.sync_info
                if si is not None and len(si.on_wait) > 1:
                    waits = list(si.on_wait)
                    for k, w in enumerate(waits[:-1]):
                        nop = mybir.InstNoOp(name=f"{inst.name}_wsplit{k}")
                        nop.engine = inst.engine
                        nop.sync_info = bass_rust.SyncInfo(on_wait=[w], on_update=[])
                        nc.register_instruction(nop, overwrite=True)
                        out.append(nop)
                    inst.sync_info = bass_rust.SyncInfo(
                        on_wait=[waits[-1]], on_update=list(si.on_update))
                    changed = True
                out.append(inst)
            if changed:
                bb.instructions = out


def _install_drain_patch():
    from concourse import tile as _tile
    if getattr(_tile.TileContext, '_drain_patched', False):
        return

    def _drain_and_barrier(self, tick_clock, wait_clock):
        probe = self.nc.sync.nop(nofuse=True)
        wait_clock.add_sem_waits(
            probe.ins, ScopedClock({None: tick_clock.global_clock}))
        si = probe.ins.sync_info
        if si is not None and len(si.on_wait) > 1:
            waits = list(si.on_wait)
            probe.ins.sync_info = bass_rust.SyncInfo(
                on_wait=[waits[0]], on_update=list(si.on_update))
            for w in waits[1:]:
                extra = self.nc.sync.nop(nofuse=True)
                extra.ins.sync_info = bass_rust.SyncInfo(on_wait=[w], on_update=[])
        self.nc.sync.drain()
        self.nc.all_engine_barrier()
        assert self.sems is not None
        popped = self.nc._tile_sem_poison_stack.pop()
        assert popped is self._sem_poison
        self.nc.clear_and_free_semaphores(list(self.sems.allocated().values()))
        self.nc.all_engine_barrier()

    _tile.TileContext._drain_and_barrier = _drain_and_barrier
    _tile.TileContext._drain_patched = True


_install_drain_patch()


# --------------------------------------------------------------------------
# Host-side prep
# --------------------------------------------------------------------------
def _assign_basins(basin_index, n_basins):
    """Greedy bin-pack basins onto cores, balancing unit counts."""
    counts = np.bincount(basin_index, minlength=n_basins)
    order = np.argsort(-counts)
    loads = [0] * NCORES
    core_of_basin = np.zeros(n_basins, np.int32)
    basins_of_core = [[] for _ in range(NCORES)]
    for b in order:
        c = int(np.argmin(loads))
        core_of_basin[b] = c
        basins_of_core[c].append(int(b))
        loads[c] += int(counts[b])
    assert max(loads) <= U, f"core overload {loads}"
    assert max(len(bs) for bs in basins_of_core) <= B_PAD
    return core_of_basin, basins_of_core


def _scan_layout(arr_u_t, T):
    """[U, T] -> [P, G*T] group-major (unit = g*128 + p)."""
    return arr_u_t.reshape(G, P, T).transpose(1, 0, 2).reshape(P, G * T)


def _prep_core(inp, unit_ids, basins, T):
    """Build the per-core input map. unit_ids: global unit indices (n<=U)."""
    n = len(unit_ids)
    pad = U - n
    pick = lambda a: np.concatenate([a[unit_ids], np.repeat(a[:1] * 0, pad, 0)], 0) if pad else a[unit_ids]

    x_dyn = pick(inp['x_dyn']).astype(f32)          # [U, T, 8]
    x_static = pick(inp['x_static']).astype(f32)    # [U, 16]
    prcp = pick(inp['prcp']).astype(f32)
    tavg = pick(inp['tavg']).astype(f32)
    doyf = pick(inp['doy']).astype(f32)
    elev = pick(inp['elev_m']).astype(f32)
    lat = pick(inp['lat_rad']).astype(f32)
    aw = pick(inp['area_weight']).astype(f32)
    if pad:
        aw[n:] = 0.0
    bidx = pick(inp['basin_index'])

    # LSTM x stream [T, 17, 256]: rows 0:8 x for A units, 8:16 B, 16 ones
    xh = np.zeros((T, 17, 256), f32)
    xh[:, 0:8, :] = x_dyn[0:256].transpose(1, 2, 0)
    xh[:, 8:16, :] = x_dyn[256:512].transpose(1, 2, 0)
    xh[:, 16, :] = 1.0

    # static encoder rhs [34, 256]: rows 0:16 xs A, 16 ones@16? layout below
    xs2 = np.zeros((34, 256), f32)
    xs2[0:16, :] = x_static[0:256].T
    xs2[16, :] = 1.0
    xs2[17:33, :] = x_static[256:512].T
    xs2[33, :] = 1.0

    Wih, Whh, bl = inp['Wih'].astype(f32), inp['Whh'].astype(f32), inp['b_lstm'].astype(f32)
    Ws, bs = inp['Ws'].astype(f32), inp['bs'].astype(f32)
    Wout, bout = inp['Wout'].astype(f32), inp['bout'].astype(f32)

    # gate weights: reference order i,f,g,o
    Lh = np.zeros((4, 128, 128), f32)
    Lx = np.zeros((4, 17, 128), f32)
    for k in range(4):
        Wg = Whh[k * HID:(k + 1) * HID]     # [64, 64]
        Xg = Wih[k * HID:(k + 1) * HID]     # [64, 8]
        bg = bl[k * HID:(k + 1) * HID]
        Lh[k, 0:64, 0:64] = Wg.T
        Lh[k, 64:128, 64:128] = Wg.T
        Lx[k, 0:8, 0:64] = Xg.T
        Lx[k, 8:16, 64:128] = Xg.T
        Lx[k, 16, 0:64] = bg
        Lx[k, 16, 64:128] = bg
    # static encoder lhsT [34, 128]
    Ls = np.zeros((34, 128), f32)
    Ls[0:16, 0:64] = Ws.T
    Ls[16, 0:64] = bs
    Ls[17:33, 64:128] = Ws.T
    Ls[33, 64:128] = bs
    # output head lhsT [128, 64] x2 (h-part, s-part); col m<30 A, 30..59 B
    Lo_h = np.zeros((128, 64), f32)
    Lo_s = np.zeros((128, 64), f32)
    Lo_h[0:64, 0:30] = Wout[:, 0:HID].T
    Lo_h[64:128, 30:60] = Wout[:, 0:HID].T
    Lo_s[0:64, 0:30] = Wout[:, HID:2 * HID].T
    Lo_s[64:128, 30:60] = Wout[:, HID:2 * HID].T
    bout2 = np.zeros((64, 1), f32)
    bout2[0:30, 0] = bout
    bout2[30:60, 0] = bout
    lo2 = np.zeros((64, 1), f32)
    span2 = np.zeros((64, 1), f32)
    for k, (_, lo_, hi_) in enumerate(PARAM_SPECS):
        lo2[k, 0] = lo_
        span2[k, 0] = hi_ - lo_
        lo2[30 + k, 0] = lo_
        span2[30 + k, 0] = hi_ - lo_

    # met arrays in scan layout
    met = np.stack([_scan_layout(prcp, T), _scan_layout(tavg, T),
                    _scan_layout(doyf, T)], 0)       # [3, P, G*T]
    ev4 = elev.reshape(G, P).T.copy()                # [P, G]
    lat4 = lat.reshape(G, P).T.copy()

    # one-hot segment matrix with area weight folded [P, G*B_PAD]
    woh = np.zeros((P, G * B_PAD), f32)
    bmap = {b: i for i, b in enumerate(basins)}
    for u in range(n):
        b = bmap[int(bidx[u])]
        g, p_ = u // P, u % P
        woh[p_, g * B_PAD + b] = aw[u]

    ident = np.eye(128, dtype=f32)

    return {
        'xh': xh.reshape(T * 17, 256), 'xs2': xs2,
        'Lh': Lh.reshape(4 * 128, 128), 'Lx': Lx.reshape(4 * 17, 128),
        'Ls': Ls, 'Lo_h': Lo_h, 'Lo_s': Lo_s,
        'bout2': bout2, 'lo2': lo2, 'span2': span2,
        'met': met.reshape(3 * P, G * T), 'ev4': ev4, 'lat4': lat4,
        'woh': woh, 'ident': ident,
    }


# --------------------------------------------------------------------------
# Kernel builder
# --------------------------------------------------------------------------
def build_kernel(T=730, debug_taps=()):
    nc = bass.Bass("TRN2")
    # ACT float biases must exist as const APs
    for val in (-1.405, float(np.pi / 2)):
        _t = nc.alloc_sbuf_tensor(f"constf32_{abs(hash(val)) % 99999}", [128, 1], DT)
        nc.gpsimd.memset(_t.ap(), val)
        nc.const_aps.aps[(DT, val)] = _t.ap()
    nc.all_engine_barrier()
    TP = T + 1
    GT = G * T

    dram = {}
    def din(name, shape):
        dram[name] = nc.dram_tensor(name, list(shape), DT, kind="ExternalInput")
        return dram[name]

    def dout(name, shape):
        dram[name] = nc.dram_tensor(name, list(shape), DT, kind="ExternalOutput")
        return dram[name]

    din('xh', [T * 17, 256])
    din('xs2', [34, 256])
    din('Lh', [4 * 128, 128])
    din('Lx', [4 * 17, 128])
    din('Ls', [34, 128])
    din('Lo_h', [128, 64])
    din('Lo_s', [128, 64])
    din('bout2', [64, 1])
    din('lo2', [64, 1])
    din('span2', [64, 1])
    din('met', [3 * P, GT])
    din('ev4', [P, G])
    din('lat4', [P, G])
    din('woh', [P, G * B_PAD])
    din('ident', [128, 128])

    dout('q_unit', [P, GT])
    dout('qg', [B_PAD, T])
    dout('qb', [B_PAD, T])
    taps = {}
    for name, shape in debug_taps:
        taps[name] = dout(name, shape)

    dt_inc = 1.0 / N_INC

    with TileContext(nc) as tc:
        # ------------------- persistent pools -------------------
        cpool = tc.alloc_tile_pool(name="consts", bufs=1)
        spool = tc.alloc_tile_pool(name="streams", bufs=1)

        def ctile(tag, shape=(P, G)):
            return cpool.tile(list(shape), DT, tag=tag, name=tag)

        # ==================== PHASE 1: LSTM =====================
        with tc.tile_pool(name="lstm", bufs=1) as lp, \
             tc.tile_pool(name="lstm_lw", bufs=2) as lwp, \
             tc.tile_pool(name="lstm_ps", bufs=1, space="PSUM") as pp:
            h = lp.tile([128, 256], DT, tag="h")
            c = lp.tile([128, 256], DT, tag="c")
            Lh_t = lp.tile([128, 4, 128], DT, tag="Lh")
            Lx_t = lp.tile([17, 4, 128], DT, tag="Lx")
            nc.vector.memset(h[:], 0.0)
            nc.vector.memset(c[:], 0.0)
            nc.sync.dma_start(Lh_t[:], dram['Lh'][:].rearrange("(k a) b -> a k b", k=4))
            nc.sync.dma_start(Lx_t[:], dram['Lx'][:].rearrange("(k a) b -> a k b", k=4))
            psum3 = pp.tile([128, 1536], DT, tag="ps3")
            psumg = pp.tile([128, 512], DT, tag="psg")
            xdr = dram['xh'][:].rearrange("(t k) n -> t k n", k=17)

            def lstm_body(iv):
                x2 = lwp.tile([17, 256], DT, tag="x2")
                nc.sync.dma_start(x2[:], xdr[bass.ds(iv, 1)].rearrange("o k n -> (o k) n"))
                # gates i,f,o into psum3 thirds; g into psumg
                slots = [psum3[:, 0:256], psum3[:, 512:768], psum3[:, 1024:1280],
                         psumg[:, 0:256]]
                for j, k in enumerate((0, 1, 3, 2)):   # i,f,o,g (k = ref gate idx)
                    nc.tensor.matmul(slots[j], Lh_t[:, k], h[:], start=True, stop=False)
                    nc.tensor.matmul(slots[j], Lx_t[:, k], x2[:], start=False, stop=True)
                sig3 = lwp.tile([128, 768], DT, tag="sig3")
                tg = lwp.tile([128, 256], DT, tag="tg")
                th = lwp.tile([128, 256], DT, tag="th")
                t1 = lwp.tile([128, 256], DT, tag="t1")
                ps3v = psum3[:].rearrange("p (b n) -> p b n", n=512)[:, :, 0:256]
                nc.scalar.activation(sig3[:].rearrange("p (b n) -> p b n", n=256), ps3v, AF.Sigmoid)
                nc.scalar.activation(tg[:], psumg[:, 0:256], AF.Tanh)
                nc.vector.tensor_tensor(t1[:], sig3[:, 0:256], tg[:], AOT.mult)
                nc.vector.tensor_tensor(c[:], sig3[:, 256:512], c[:], AOT.mult)
                nc.vector.tensor_tensor(c[:], c[:], t1[:], AOT.add)
                nc.scalar.activation(th[:], c[:], AF.Tanh)
                nc.vector.tensor_tensor(h[:], sig3[:, 512:768], th[:], AOT.mult)

            with tc.For_i(0, T, 1) as iv:
                lstm_body(iv)

            # ---- params head ----
            xs2_t = lp.tile([34, 256], DT, tag="xs2")
            Ls_t = lp.tile([34, 128], DT, tag="Ls")
            Lo_h_t = lp.tile([128, 64], DT, tag="Loh")
            Lo_s_t = lp.tile([128, 64], DT, tag="Los")
            b3 = lp.tile([64, 3], DT, tag="b3")  # cols: bout2, lo2, span2
            nc.sync.dma_start(xs2_t[:], dram['xs2'][:])
            nc.sync.dma_start(Ls_t[:], dram['Ls'][:])
            nc.sync.dma_start(Lo_h_t[:], dram['Lo_h'][:])
            nc.sync.dma_start(Lo_s_t[:], dram['Lo_s'][:])
            nc.sync.dma_start(b3[:, 0:1], dram['bout2'][:])
            nc.sync.dma_start(b3[:, 1:2], dram['lo2'][:])
            nc.sync.dma_start(b3[:, 2:3], dram['span2'][:])
            s_pk = lp.tile([128, 256], DT, tag="spk")
            nc.tensor.matmul(psumg[:, 0:256], Ls_t[:], xs2_t[:], start=True, stop=True)
            nc.scalar.activation(s_pk[:], psumg[:, 0:256], AF.Tanh)
            praw = pp.tile([64, 512], DT, tag="praw")
            nc.tensor.matmul(praw[:, 0:256], Lo_h_t[:], h[:], start=True, stop=False)
            nc.tensor.matmul(praw[:, 0:256], Lo_s_t[:], s_pk[:], start=False, stop=True)
            gsig = lp.tile([64, 256], DT, tag="gsig")
            nc.scalar.activation(gsig[:], praw[:, 0:256], AF.Sigmoid, bias=b3[:, 0:1])
            nc.vector.tensor_scalar(out=gsig[:], in0=gsig[:], scalar1=b3[:, 2:3],
                                    scalar2=b3[:, 1:2], op0=AOT.mult, op1=AOT.add)
            # transpose each 128-unit chunk: [60,128] -> [128,60]
            ident_t = lp.tile([128, 128], DT, tag="ident")
            nc.sync.dma_start(ident_t[:], dram['ident'][:])
            P_all = cpool.tile([P, G, 32], DT, tag="P_all")
            nc.vector.memset(P_all[:], 0.0)
            ptp = pp.tile([128, 128], DT, tag="ptp")
            for ch in range(2):
                nc.tensor.transpose(ptp[:, 0:64], gsig[:, ch * 128:(ch + 1) * 128], ident_t[0:64, 0:64])
                nc.scalar.activation(P_all[:, ch, 0:30], ptp[:, 0:30], AF.Copy)
                nc.scalar.activation(P_all[:, ch + 2, 0:30], ptp[:, 30:60], AF.Copy)
            if 'tap_params' in taps:
                nc.sync.dma_start(taps['tap_params'][:],
                                  P_all[:].rearrange("p g k -> p (g k)"))

        def pk(name):
            return P_all[:, :, PIDX[name]]

        # ============= PHASE 2: derived constants [P,G] =============
        TT = nc.vector.tensor_tensor
        TS = nc.vector.tensor_scalar
        STT = nc.vector.scalar_tensor_tensor
        ACTV = nc.scalar.activation

        ev_t = ctile("ev4")
        lat_t = ctile("lat4")
        nc.sync.dma_start(ev_t[:], dram['ev4'][:])
        nc.sync.dma_start(lat_t[:], dram['lat4'][:])

        def new_c(tag):
            return ctile(tag)

        # snow consts
        pa_fac = new_c("pa_fac"); ACTV(pa_fac[:], ev_t[:], AF.Exp, scale=-1.0 / 8434.0)
        mf_a = new_c("mf_a"); TT(mf_a[:], pk('MFMAX'), pk('MFMIN'), AOT.add); TS(out=mf_a[:], in0=mf_a[:], scalar1=0.5, scalar2=None, op0=AOT.mult)
        mf_b = new_c("mf_b"); TT(mf_b[:], pk('MFMAX'), pk('MFMIN'), AOT.subtract); TS(out=mf_b[:], in0=mf_b[:], scalar1=0.5, scalar2=None, op0=AOT.mult)
        one_m_tipm = new_c("omt"); TS(out=one_m_tipm[:], in0=pk('TIPM'), scalar1=-1.0, scalar2=1.0, op0=AOT.mult, op1=AOT.add)
        mros_k = new_c("mros_k"); TT(mros_k[:], pk('UADJ'), pa_fac[:], AOT.mult); TS(out=mros_k[:], in0=mros_k[:], scalar1=0.0125, scalar2=None, op0=AOT.mult)
        # pet consts
        coef29 = new_c("coef29"); TS(out=coef29[:], in0=pk('HAMON_COEF'), scalar1=29.8 * 0.6108, scalar2=None, op0=AOT.mult)
        sl = new_c("sinlat"); ACTV(sl[:], lat_t[:], AF.Sin)
        cl = new_c("coslat"); ACTV(cl[:], lat_t[:], AF.Sin, bias=float(np.pi / 2))
        rcl = new_c("rcoslat"); nc.vector.reciprocal(rcl[:], cl[:])
        ntanlat = new_c("ntanlat"); TT(ntanlat[:], sl[:], rcl[:], AOT.mult); TS(out=ntanlat[:], in0=ntanlat[:], scalar1=-1.0, scalar2=None, op0=AOT.mult)
        # sacsma consts
        inv_uztwm = new_c("inv_uztwm"); nc.vector.reciprocal(inv_uztwm[:], pk('UZTWM'))
        c1 = new_c("c1"); TS(out=c1[:], in0=pk('LZPK'), scalar1=-dt_inc, scalar2=1.0, op0=AOT.mult, op1=AOT.add)
        c2 = new_c("c2"); TS(out=c2[:], in0=pk('LZSK'), scalar1=-dt_inc, scalar2=1.0, op0=AOT.mult, op1=AOT.add)
        c3 = new_c("c3"); TS(out=c3[:], in0=pk('UZK'), scalar1=-dt_inc, scalar2=1.0, op0=AOT.mult, op1=AOT.add)
        decay3 = cpool.tile([P, 12], DT, tag="decay3")
        nc.vector.tensor_copy(decay3[:, 0:4], c1[:])
        nc.vector.tensor_copy(decay3[:, 4:8], c2[:])
        nc.vector.tensor_copy(decay3[:, 8:12], c3[:])
        inv_uzfwm = new_c("inv_uzfwm"); nc.vector.reciprocal(inv_uzfwm[:], pk('UZFWM'))
        kperc = new_c("kperc")
        tmpc = cpool.tile([P, G], DT, tag="tmpc")
        TT(kperc[:], pk('LZFPM'), pk('LZPK'), AOT.mult)
        TT(tmpc[:], pk('LZFSM'), pk('LZSK'), AOT.mult)
        TT(kperc[:], kperc[:], tmpc[:], AOT.add)
        TT(kperc[:], kperc[:], inv_uzfwm[:], AOT.mult)
        TS(out=kperc[:], in0=kperc[:], scalar1=dt_inc, scalar2=None, op0=AOT.mult)
        lzmax = new_c("lzmax"); TT(lzmax[:], pk('LZTWM'), pk('LZFPM'), AOT.add); TT(lzmax[:], lzmax[:], pk('LZFSM'), AOT.add)
        Bt = new_c("Bt")
        TT(Bt[:], kperc[:], pk('ZPERC'), AOT.mult)
        ACTV(Bt[:], Bt[:], AF.Ln)
        ACTV(tmpc[:], lzmax[:], AF.Ln)
        TT(tmpc[:], tmpc[:], pk('REXP'), AOT.mult)
        TT(Bt[:], Bt[:], tmpc[:], AOT.subtract)
        UL = new_c("UL"); TT(UL[:], pk('UZTWM'), pk('LZTWM'), AOT.add)
        inv_UL = new_c("inv_UL"); nc.vector.reciprocal(inv_UL[:], UL[:])
        pfreec = new_c("pfreec"); TS(out=pfreec[:], in0=pk('PFREE'), scalar1=-1.0, scalar2=1.0, op0=AOT.mult, op1=AOT.add)
        kq = new_c("kq"); TS(out=kq[:], in0=pk('UZK'), scalar1=dt_inc, scalar2=None, op0=AOT.mult)
        inv_side = new_c("inv_side"); TS(out=inv_side[:], in0=pk('SIDE'), scalar1=1.0, scalar2=None, op0=AOT.add); nc.vector.reciprocal(inv_side[:], inv_side[:])
        kp_side = new_c("kp_side"); TT(kp_side[:], pk('LZPK'), inv_side[:], AOT.mult); TS(out=kp_side[:], in0=kp_side[:], scalar1=dt_inc, scalar2=None, op0=AOT.mult)
        ks_side = new_c("ks_side"); TT(ks_side[:], pk('LZSK'), inv_side[:], AOT.mult); TS(out=ks_side[:], in0=ks_side[:], scalar1=dt_inc, scalar2=None, op0=AOT.mult)
        pinc_k = new_c("pinc_k"); TT(pinc_k[:], pk('PCTIM'), pk('ADIMP'), AOT.add); TS(out=pinc_k[:], in0=pinc_k[:], scalar1=-dt_inc, scalar2=dt_inc, op0=AOT.mult, op1=AOT.add)
        # routing consts
        nm1 = new_c("nm1"); TS(out=nm1[:], in0=pk('UH_N'), scalar1=-1.0, scalar2=None, op0=AOT.add)
        inv_tau = new_c("inv_tau"); nc.vector.reciprocal(inv_tau[:], pk('UH_TAU'))
        uh = cpool.tile([P, UH_LEN, G], DT, tag="uh")
        wsum = new_c("wsum")
        for l in range(1, UH_LEN + 1):
            wl = uh[:, l - 1, :]
            TS(out=tmpc[:], in0=inv_tau[:], scalar1=float(-l), scalar2=None, op0=AOT.mult)
            STT(out=wl, in0=nm1[:], scalar=float(np.log(l)), in1=tmpc[:], op0=AOT.mult, op1=AOT.add)
            ACTV(wl, wl, AF.Exp)
            if l == 1:
                nc.vector.tensor_copy(wsum[:], wl)
            else:
                TT(wsum[:], wsum[:], wl, AOT.add)
        nc.vector.reciprocal(wsum[:], wsum[:])
        for l in range(UH_LEN):
            TT(uh[:, l, :], uh[:, l, :], wsum[:], AOT.mult)

        # per-unit state init constants
        half = lambda name, tag: (lambda t_: (TS(out=t_[:], in0=pk(name), scalar1=0.5, scalar2=None, op0=AOT.mult), t_)[1])(ctile(tag))

        # ============= PHASE 3: PET + SNOW17 (big arrays) =============
        def bc(tile_pg, n=T):
            """broadcast [P,G] const along time -> [P,G,n] AP."""
            ap = tile_pg if isinstance(tile_pg, bass.AP) else tile_pg[:]
            return ap[:, :, None].broadcast_to((P, G, n))

        def g3(arr):
            ap = arr if isinstance(arr, bass.AP) else arr[:]
            return ap.rearrange("p (g t) -> p g t", g=G)

        met_t = spool.tile([P, 3, GT], DT, tag="met")
        nc.sync.dma_start(met_t[:], dram['met'][:].rearrange("(k p) n -> p k n", k=3))
        prcp_a = met_t[:, 0]
        tavg_a = met_t[:, 1]
        doyf_a = met_t[:, 2]

        e_a = spool.tile([P, GT], DT, tag="e_a")        # PET
        euz_a = spool.tile([P, GT], DT, tag="euz_a")
        roi_a = spool.tile([P, GT], DT, tag="roi_a")
        pad_a = spool.tile([P, GT], DT, tag="pad_a")
        pin_a = spool.tile([P, GT], DT, tag="pin_a")
        surf_a = spool.tile([P, GT], DT, tag="surf_a")
        base_a = spool.tile([P, GT], DT, tag="base_a")

        with tc.tile_pool(name="snow", bufs=1) as sp:
            w1 = sp.tile([P, GT], DT, tag="w1", name="w1")
            w2 = sp.tile([P, GT], DT, tag="w2", name="w2")
            # scratch aliased onto stream buffers that are only written later
            w3 = surf_a
            w4 = base_a
            w5 = pin_a
            # ---- PET into e_a ----
            # decl' = sin(2pi/365*doy - 1.405); decl = 0.4093*decl'
            # range-reduce the sin argument into [-pi, pi]
            TS(out=w1[:], in0=doyf_a, scalar1=float(2 * np.pi / 365), scalar2=-1.405,
               op0=AOT.mult, op1=AOT.add)
            TS(out=w4[:], in0=w1[:], scalar1=float(np.pi), scalar2=None, op0=AOT.is_gt)
            STT(out=w1[:], in0=w4[:], scalar=float(-2 * np.pi), in1=w1[:],
                op0=AOT.mult, op1=AOT.add)
            ACTV(w1[:], w1[:], AF.Sin)
            TS(out=w1[:], in0=w1[:], scalar1=0.4093, scalar2=None, op0=AOT.mult)
            # tan(decl) = sin/cos
            ACTV(w2[:], w1[:], AF.Sin)
            ACTV(w3[:], w1[:], AF.Sin, bias=float(np.pi / 2))
            nc.vector.reciprocal(w3[:], w3[:])
            TT(w2[:], w2[:], w3[:], AOT.mult)
            # cosw = clip(-tanlat*tan(decl))
            TT(g3(w2), g3(w2), bc(ntanlat), AOT.mult)
            TS(out=w2[:], in0=w2[:], scalar1=0.9999, scalar2=-0.9999, op0=AOT.min, op1=AOT.max)
            # daylen*24/pi = 12 - atan(cosw*rsqrt(1-cosw^2))*(24/pi)
            ACTV(w3[:], w2[:], AF.Square)
            TS(out=w3[:], in0=w3[:], scalar1=-1.0, scalar2=1.0, op0=AOT.mult, op1=AOT.add)
            ACTV(w3[:], w3[:], AF.Sqrt)
            nc.vector.reciprocal(w3[:], w3[:])
            TT(w2[:], w2[:], w3[:], AOT.mult)
            ACTV(w2[:], w2[:], AF.Arctan)
            TS(out=w2[:], in0=w2[:], scalar1=float(-24.0 / np.pi), scalar2=12.0, op0=AOT.mult, op1=AOT.add)
            # esat' = exp(17.27*T/(T+237.3))
            TS(out=w3[:], in0=tavg_a, scalar1=237.3, scalar2=None, op0=AOT.add)
            nc.vector.reciprocal(w3[:], w3[:])
            TT(w3[:], w3[:], tavg_a, AOT.mult)
            ACTV(w3[:], w3[:], AF.Exp, scale=17.27)
            # 1/(T+273.2)
            TS(out=w4[:], in0=tavg_a, scalar1=273.2, scalar2=None, op0=AOT.add)
            nc.vector.reciprocal(w4[:], w4[:])
            TT(w2[:], w2[:], w3[:], AOT.mult)
            TT(w2[:], w2[:], w4[:], AOT.mult)
            TT(g3(e_a), g3(w2), bc(coef29), AOT.mult)
            if 'tap_pet' in taps:
                nc.sync.dma_start(taps['tap_pet'][:], e_a[:])

            # ---- SNOW17 ----
            # w1 = m (is_snow), w2 = pm, w3 = ps, w4 = prain
            pxt = ctile("pxt"); nc.vector.tensor_copy(pxt[:], pk('PXTEMP'))
            scf = ctile("scf"); nc.vector.tensor_copy(scf[:], pk('SCF'))
            TT(g3(w1), g3(tavg_a[:]), bc(pxt), AOT.is_le)
            TT(w2[:], prcp_a, w1[:], AOT.mult)
            TT(g3(w3), g3(w2), bc(scf), AOT.mult)
            TT(w4[:], prcp_a, w2[:], AOT.subtract)
            # ati scan: b = TIPM*min(ta,0) into w5
            tipm = ctile("tipm"); nc.vector.tensor_copy(tipm[:], pk('TIPM'))
            TS(out=w5[:], in0=tavg_a, scalar1=0.0, scalar2=None, op0=AOT.min)
            TT(g3(w5), g3(w5), bc(tipm), AOT.mult)
            ati = w2  # reuse pm slot? pm no longer needed (ps/prain done)
            for g in range(G):
                nc.vector.tensor_tensor_scan(
                    ati[:, g * T:(g + 1) * T],
                    one_m_tipm[:, g:g + 1].broadcast_to((P, T)),
                    w5[:, g * T:(g + 1) * T], 0.0, AOT.mult, AOT.add)
            # M into w5: mf*relu(ta-MBASE) + mros - NMF*relu(-ati)
            mbase = ctile("mbase"); nc.vector.tensor_copy(mbase[:], pk('MBASE'))
            nmf = ctile("nmf"); nc.vector.tensor_copy(nmf[:], pk('NMF'))
            daygm = ctile("daygm"); nc.vector.tensor_copy(daygm[:], pk('DAYGM'))
            plwhc = ctile("plwhc"); nc.vector.tensor_copy(plwhc[:], pk('PLWHC'))
            mtmp = pad_a
            mtmp2_pre = roi_a
            TS(out=mtmp[:], in0=doyf_a, scalar1=float(2 * np.pi / 366), scalar2=None,
               op0=AOT.mult)
            TS(out=mtmp2_pre[:], in0=mtmp[:], scalar1=float(np.pi), scalar2=None, op0=AOT.is_gt)
            STT(out=mtmp[:], in0=mtmp2_pre[:], scalar=float(-2 * np.pi), in1=mtmp[:],
                op0=AOT.mult, op1=AOT.add)
            ACTV(mtmp[:], mtmp[:], AF.Sin)
            TT(g3(mtmp), g3(mtmp), bc(mf_b), AOT.mult)
            TT(g3(mtmp), g3(mtmp), bc(mf_a), AOT.add)          # mf
            mtmp2 = roi_a
            TT(g3(mtmp2), g3(tavg_a[:]), bc(mbase), AOT.subtract)
            TS(out=mtmp2[:], in0=mtmp2[:], scalar1=0.0, scalar2=None, op0=AOT.max)
            TT(mtmp[:], mtmp[:], mtmp2[:], AOT.mult)           # mf*relu(ta-MBASE)
            TS(out=mtmp2[:], in0=tavg_a, scalar1=0.0, scalar2=None, op0=AOT.max)
            TT(mtmp2[:], mtmp2[:], w4[:], AOT.mult)
            TT(g3(mtmp2), g3(mtmp2), bc(mros_k), AOT.mult)     # mros
            TT(mtmp[:], mtmp[:], mtmp2[:], AOT.add)
            TS(out=mtmp2[:], in0=ati[:], scalar1=-1.0, scalar2=0.0, op0=AOT.mult, op1=AOT.max)
            TT(g3(mtmp2), g3(mtmp2), bc(nmf), AOT.mult)
            TT(mtmp[:], mtmp[:], mtmp2[:], AOT.subtract)
            TS(out=w5[:], in0=mtmp[:], scalar1=0.0, scalar2=None, op0=AOT.max)  # M
            # cwi = ps - M - DAYGM into mtmp
            TT(mtmp[:], w3[:], w5[:], AOT.subtract)
            TT(g3(mtmp), g3(mtmp), bc(daygm), AOT.subtract)
            # wi scan (with lead zero col per group)
            wi3 = sp.tile([P, G * TP], DT, tag="wi3")
            zcol = cpool.tile([P, 1], DT, tag="zcol")
            nc.vector.memset(zcol[:], 0.0)
            for g in range(G):
                nc.vector.memset(wi3[:, g * TP:g * TP + 1], 0.0)
                nc.vector.tensor_tensor_scan(
                    wi3[:, g * TP + 1:(g + 1) * TP],
                    mtmp[:, g * T:(g + 1) * T],
                    zcol[:].broadcast_to((P, T)), 0.0, AOT.add, AOT.max)
            wi3v = wi3[:].rearrange("p (g t) -> p g t", g=G)
            wi_prev = wi3v[:, :, 0:T]
            wi_cur = wi3v[:, :, 1:TP]
            # wi1 = wi_prev + ps -> w3 (consumes ps)
            TT(g3(w3), wi_prev, g3(w3), AOT.add)
            # melt = min(M, wi1) -> mtmp
            TT(mtmp[:], w5[:], w3[:], AOT.min)
            # wi2 = relu(wi1 - M) -> w3
            TT(w3[:], w3[:], w5[:], AOT.subtract)
            TS(out=w3[:], in0=w3[:], scalar1=0.0, scalar2=None, op0=AOT.max)
            # gm = min(wi2, DAYGM) -> w3 (wi2 consumed)
            TT(g3(w3), g3(w3), bc(daygm), AOT.min)
            # rop = prain * (wi_cur > 0) -> mtmp2
            rop = mtmp2
            TS3 = euz_a
            # mask = wi_cur > 0
            nc.vector.tensor_scalar(out=g3(TS3), in0=wi_cur, scalar1=0.0, scalar2=None, op0=AOT.is_gt)
            TT(rop[:], w4[:], TS3[:], AOT.mult)
            # rain_free = prain - rop -> w4
            TT(w4[:], w4[:], rop[:], AOT.subtract)
            # m_t = melt + rop -> mtmp
            TT(mtmp[:], mtmp[:], rop[:], AOT.add)
            # h_t = PLWHC * wi_cur -> w5
            TT(g3(w5), wi_cur, bc(plwhc), AOT.mult)
            # wq scan
            wq2 = sp.tile([P, G * TP], DT, tag="wq2")
            for g in range(G):
                nc.vector.memset(wq2[:, g * TP:g * TP + 1], 0.0)
                nc.vector.tensor_tensor_scan(
                    wq2[:, g * TP + 1:(g + 1) * TP],
                    mtmp[:, g * T:(g + 1) * T],
                    w5[:, g * T:(g + 1) * T], 0.0, AOT.add, AOT.min)
            wq2v = wq2[:].rearrange("p (g t) -> p g t", g=G)
            # out = (wq_prev + m_t) - wq_cur -> mtmp
            TT(g3(mtmp), wq2v[:, :, 0:T], g3(mtmp), AOT.add)
            TT(g3(mtmp), g3(mtmp), wq2v[:, :, 1:TP], AOT.subtract)
            # eff = out + gm + rain_free -> w1 (pav)
            TT(w1[:], mtmp[:], w3[:], AOT.add)
            TT(w1[:], w1[:], w4[:], AOT.add)
            if 'tap_eff' in taps:
                nc.sync.dma_start(taps['tap_eff'][:], w1[:])

            # ---- sacsma streams ----
            TT(g3(euz_a), g3(e_a[:]), bc(inv_uztwm), AOT.mult)
            pctim = ctile("pctim"); nc.vector.tensor_copy(pctim[:], pk('PCTIM'))
            adimp = ctile("adimp"); nc.vector.tensor_copy(adimp[:], pk('ADIMP'))
            TT(g3(roi_a), g3(w1), bc(pctim), AOT.mult)
            TT(g3(pad_a), g3(w1), bc(adimp), AOT.mult)
            TT(g3(pin_a), g3(w1), bc(pinc_k), AOT.mult)

        # ============= PHASE 4: SAC-SMA day loop =============
        statep = tc.alloc_tile_pool(name="sac_state", bufs=1)
        uztwc = statep.tile([P, G], DT, tag="uztwc")
        lztwc = statep.tile([P, G], DT, tag="lztwc")
        adimc = statep.tile([P, G], DT, tag="adimc")
        S3 = statep.tile([P, 12], DT, tag="S3")   # lzfpc | lzfsc | uzfwc
        lzfpc = S3[:, 0:4]
        lzfsc = S3[:, 4:8]
        uzfwc = S3[:, 8:12]
        TS(out=uztwc[:], in0=pk('UZTWM'), scalar1=0.5, scalar2=None, op0=AOT.mult)
        TS(out=lztwc[:], in0=pk('LZTWM'), scalar1=0.5, scalar2=None, op0=AOT.mult)
        TS(out=adimc[:], in0=UL[:], scalar1=0.5, scalar2=None, op0=AOT.mult)
        TS(out=lzfpc, in0=pk('LZFPM'), scalar1=0.5, scalar2=None, op0=AOT.mult)
        TS(out=lzfsc, in0=pk('LZFSM'), scalar1=0.5, scalar2=None, op0=AOT.mult)
        TS(out=uzfwc, in0=pk('UZFWM'), scalar1=0.5, scalar2=None, op0=AOT.mult)

        e3v, euz3, roi3, pad3, pin3 = (g3(a) for a in (e_a, euz_a, roi_a, pad_a, pin_a))
        surf3, base3 = g3(surf_a), g3(base_a)

        lztwm_c = ctile("lztwm_c"); nc.vector.tensor_copy(lztwm_c[:], pk('LZTWM'))
        lzfpm_c = ctile("lzfpm_c"); nc.vector.tensor_copy(lzfpm_c[:], pk('LZFPM'))
        lzfsm_c = ctile("lzfsm_c"); nc.vector.tensor_copy(lzfsm_c[:], pk('LZFSM'))
        uztwm_c = ctile("uztwm_c"); nc.vector.tensor_copy(uztwm_c[:], pk('UZTWM'))
        uzfwm_c = ctile("uzfwm_c"); nc.vector.tensor_copy(uzfwm_c[:], pk('UZFWM'))
        rexp_c = ctile("rexp_c"); nc.vector.tensor_copy(rexp_c[:], pk('REXP'))

        with tc.tile_pool(name="sac_scr", bufs=1) as scr:
            def st(tag, w=4):
                return scr.tile([P, w], DT, tag=tag, name=tag)

            ecol, euzc, roic, padc, pinc = st("ecol"), st("euzc"), st("roic"), st("padc"), st("pinc")
            surf, base = st("surf"), st("base")
            acc3 = st("acc3", 12)
            a1, te1, red, red2, t_a, t_b, t_c = (st(x) for x in ("a1", "te1", "red", "red2", "t_a", "t_b", "t_c"))
            lzdef, lnd, pf2, perc, pfree, dp, ds_, rden = (st(x) for x in ("lzdef", "lnd", "pf2", "perc", "pfree", "dp", "ds_", "rden"))

            def day_body(iv):
                CP = nc.vector.tensor_copy
                dsl = lambda v3: v3[:, :, bass.ds(iv, 1)].rearrange("p g o -> p (g o)")
                CP(ecol[:], dsl(e3v))
                CP(euzc[:], dsl(euz3))
                CP(roic[:], dsl(roi3))
                CP(padc[:], dsl(pad3))
                CP(pinc[:], dsl(pin3))
                # evap
                TS(out=a1[:], in0=euzc[:], scalar1=1.0, scalar2=None, op0=AOT.min)
                TT(te1[:], uztwc[:], a1[:], AOT.mult)
                TT(uztwc[:], uztwc[:], te1[:], AOT.subtract)
                TT(red[:], ecol[:], te1[:], AOT.subtract)
                TT(red2[:], red[:], uzfwc, AOT.subtract)
                TS(out=red2[:], in0=red2[:], scalar1=0.0, scalar2=None, op0=AOT.max)
                TT(t_a[:], uzfwc, red[:], AOT.subtract)
                TS(out=uzfwc, in0=t_a[:], scalar1=0.0, scalar2=None, op0=AOT.max)
                TT(t_a[:], red2[:], inv_UL[:], AOT.mult)
                TS(out=t_a[:], in0=t_a[:], scalar1=-1.0, scalar2=1.0, op0=AOT.mult, op1=AOT.add)
                TS(out=t_a[:], in0=t_a[:], scalar1=0.0, scalar2=None, op0=AOT.max)
                TT(lztwc[:], lztwc[:], t_a[:], AOT.mult)
                # adimp
                TT(adimc[:], adimc[:], padc[:], AOT.add)
                TT(t_a[:], adimc[:], inv_UL[:], AOT.mult)
                TS(out=t_a[:], in0=t_a[:], scalar1=1.0, scalar2=None, op0=AOT.min)
                TT(t_a[:], t_a[:], t_a[:], AOT.mult)
                TT(t_a[:], t_a[:], padc[:], AOT.mult)       # adsur
                TT(adimc[:], adimc[:], t_a[:], AOT.subtract)
                TT(adimc[:], adimc[:], UL[:], AOT.min)
                TT(surf[:], roic[:], t_a[:], AOT.add)
                nc.vector.memset(acc3[:], 0.0)
                for _ in range(N_INC):
                    TT(acc3[:], acc3[:], S3[:], AOT.add)
                    TT(S3[:], S3[:], decay3[:], AOT.mult)
                    TT(t_a[:], lztwc[:], lzfpc, AOT.add)
                    TT(t_a[:], t_a[:], lzfsc, AOT.add)
                    TT(lzdef[:], lzmax[:], t_a[:], AOT.subtract)
                    TS(out=lzdef[:], in0=lzdef[:], scalar1=0.0, scalar2=None, op0=AOT.max)
                    ACTV(lnd[:], lzdef[:], AF.Ln)
                    TT(t_a[:], lnd[:], rexp_c[:], AOT.mult)
                    TT(t_a[:], t_a[:], Bt[:], AOT.add)
                    ACTV(pf2[:], t_a[:], AF.Exp)
                    TT(t_a[:], pf2[:], kperc[:], AOT.add)
                    TS(out=t_a[:], in0=t_a[:], scalar1=1.0, scalar2=None, op0=AOT.min)
                    TT(t_a[:], t_a[:], uzfwc, AOT.mult)
                    TT(perc[:], t_a[:], lzdef[:], AOT.min)
                    TT(uzfwc, uzfwc, perc[:], AOT.subtract)
                    TT(t_a[:], perc[:], pfreec[:], AOT.mult)     # pc
                    TT(t_b[:], perc[:], lztwc[:], AOT.add)       # aa
                    TT(t_a[:], lztwc[:], t_a[:], AOT.add)        # lz1
                    TT(lztwc[:], t_a[:], lztwm_c[:], AOT.min)
                    TT(pfree[:], t_b[:], lztwc[:], AOT.subtract)
                    TT(dp[:], lzfpm_c[:], lzfpc, AOT.subtract)
                    TT(ds_[:], lzfsm_c[:], lzfsc, AOT.subtract)
                    TT(t_a[:], dp[:], ds_[:], AOT.add)
                    TS(out=t_a[:], in0=t_a[:], scalar1=1e-6, scalar2=None, op0=AOT.max)
                    nc.vector.reciprocal(rden[:], t_a[:])
                    TT(t_a[:], dp[:], rden[:], AOT.mult)         # fr
                    TT(t_a[:], pfree[:], t_a[:], AOT.mult)       # pf_p
                    TT(t_b[:], lzfpc, t_a[:], AOT.add)           # lzfpc1
                    TT(t_c[:], pfree[:], t_a[:], AOT.subtract)   # t4
                    TT(t_c[:], lzfsc, t_c[:], AOT.add)           # lzfsc1
                    TT(lzfpc, t_b[:], lzfpm_c[:], AOT.min)
                    TT(t_b[:], t_b[:], lzfpc, AOT.subtract)      # ex2
                    TT(t_c[:], t_c[:], t_b[:], AOT.add)          # lzfsc2
                    TT(lzfsc, t_c[:], lzfsm_c[:], AOT.min)
                    TT(t_c[:], t_c[:], lzfsc, AOT.subtract)      # ex3
                    TT(surf[:], surf[:], t_c[:], AOT.add)
                    TT(t_a[:], uztwc[:], pinc[:], AOT.add)       # uz1
                    TT(uztwc[:], t_a[:], uztwm_c[:], AOT.min)
                    TT(t_a[:], t_a[:], uztwc[:], AOT.subtract)   # dU
                    TT(t_a[:], uzfwc, t_a[:], AOT.add)           # uf1
                    TT(uzfwc, t_a[:], uzfwm_c[:], AOT.min)
                    TT(t_a[:], t_a[:], uzfwc, AOT.subtract)      # dF
                    TT(surf[:], surf[:], t_a[:], AOT.add)
                TT(t_a[:], acc3[:, 8:12], kq[:], AOT.mult)
                TT(surf[:], surf[:], t_a[:], AOT.add)
                TT(t_a[:], acc3[:, 0:4], kp_side[:], AOT.mult)
                TT(t_b[:], acc3[:, 4:8], ks_side[:], AOT.mult)
                TT(base[:], t_a[:], t_b[:], AOT.add)
                CP(dsl(surf3), surf[:])
                CP(dsl(base3), base[:])

            with tc.For_i(0, T, 1) as iv:
                day_body(iv)

        # ============= PHASE 5: routing + segment sums =============
        with tc.tile_pool(name="route", bufs=1) as rp, \
             tc.tile_pool(name="route_ps", bufs=1, space="PSUM") as rpp:
            q_a = rp.tile([P, GT], DT, tag="q_a")
            nc.vector.tensor_copy(q_a[:], base_a[:])
            for g in range(G):
                for l in range(min(UH_LEN, T)):
                    STT(out=q_a[:, g * T + l:(g + 1) * T],
                        in0=surf_a[:, g * T:(g + 1) * T - l],
                        scalar=uh[:, l, g:g + 1],
                        in1=q_a[:, g * T + l:(g + 1) * T],
                        op0=AOT.mult, op1=AOT.add)
            nc.sync.dma_start(dram['q_unit'][:], q_a[:])
            # segment sums via one-hot matmuls
            woh_t = rp.tile([P, G * B_PAD], DT, tag="woh")
            nc.sync.dma_start(woh_t[:], dram['woh'][:])
            qg_sb = rp.tile([B_PAD, T], DT, tag="qg_sb")
            qb_sb = rp.tile([B_PAD, T], DT, tag="qb_sb")
            NSPL = (T + 1) // 2
            for (src, dst) in ((q_a, qg_sb), (base_a, qb_sb)):
                ps = rpp.tile([B_PAD, 1024], DT, tag="ps_seg")
                for half_i in range(2):
                    n0 = half_i * NSPL
                    n1 = min(T, n0 + NSPL)
                    for g in range(G):
                        nc.tensor.matmul(
                            ps[:, half_i * 512:half_i * 512 + (n1 - n0)],
                            woh_t[:, g * B_PAD:(g + 1) * B_PAD],
                            src[:, g * T + n0:g * T + n1],
                            start=(g == 0), stop=(g == G - 1))
                    nc.scalar.activation(dst[:, n0:n1], ps[:, half_i * 512:half_i * 512 + (n1 - n0)], AF.Copy)
            nc.sync.dma_start(dram['qg'][:], qg_sb[:])
            nc.sync.dma_start(dram['qb'][:], qb_sb[:])

        statep.release()
        spool.release()
        cpool.release()
    _split_multi_waits(nc)
    return nc


# --------------------------------------------------------------------------
# Host orchestration
# --------------------------------------------------------------------------
_NC_CACHE = {}


def kernel(**inputs):
    T = inputs['prcp'].shape[1]
    N = inputs['prcp'].shape[0]
    nb = int(inputs['n_basins'])
    inp = {k: np.asarray(v) if hasattr(v, 'shape') else v for k, v in inputs.items()}

    basin_index = np.asarray(inp['basin_index'])
    core_of_basin, basins_of_core = _assign_basins(basin_index, nb)
    core_of_unit = core_of_basin[basin_index]

    in_maps = []
    unit_lists = []
    for c in range(NCORES):
        uids = np.where(core_of_unit == c)[0]
        # sort by basin for locality (not required, aesthetic)
        uids = uids[np.argsort(basin_index[uids], kind='stable')]
        unit_lists.append(uids)
        in_maps.append(_prep_core(inp, uids, basins_of_core[c], T))

    key = T
    if key not in _NC_CACHE:
        _NC_CACHE[key] = build_kernel(T)
    nc = _NC_CACHE[key]

    res = bass_utils.run_bass_kernel_spmd(nc, in_maps, core_ids=list(range(NCORES)))

    q_unit = np.zeros((N, T), f32)
    qg = np.zeros((nb, T), f32)
    qb = np.zeros((nb, T), f32)
    for c in range(NCORES):
        r = res.results[c]
        uids = unit_lists[c]
        qu = r['q_unit'].reshape(P, G, T).transpose(1, 0, 2).reshape(U, T)
        q_unit[uids] = qu[:len(uids)]
        for i, b in enumerate(basins_of_core[c]):
            qg[b] = r['qg'][i]
            qb[b] = r['qb'][i]
    return qg, qb, q_unit


if __name__ == '__main__':
    import reference
    inputs = reference.setup_inputs()
    out = kernel(**{k: np.asarray(v) if hasattr(v, 'shape') else v for k, v in inputs.items()})
    print([o.shape for o in out])
